# revision 2
# baseline (speedup 1.0000x reference)
"""GraphSAGE-mean (DivFeatConv) forward on 8 TRN2 NeuronCores — v3.

out = relu(feat @ W_self.T + b_self + segmean(feat[src], dst) @ W_neigh.T + b_neigh)

Strategy (SPMD, one program on 8 cores), no on-device gather at all:
  - Shard dst nodes across cores (5000/core); re-permute each core's nodes
    into 158 tiles of 32 columns via LPT degree packing so every tile has
    ~506 incident edges -> cap[t]=4 chunks of 128 edge slots (~1% padding).
  - Host stages, per core, a contiguous fp8(e4m3) edge table: chunk c holds
    the src features of its 128 edges (blob[p, c*128+d] = feat[src, d]).
    Linear DMA per supertile (16 tiles) replaces the SWDGE gather.
  - Scatter-sum onto a tile's 32 dst columns: one matmul per chunk,
    lhsT = table chunk (fp8, stationary), rhs = [128, 32] bf16 one-hot
    sel matrix (sel[e, n] = dstrel[e] == n).  Mixed fp8 x bf16 matmul
    measured ~39-43ns/chunk (LDWEIGHTS-bound).
  - All sel matrices of a supertile are built by ONE wide DVE
    tensor_tensor(is_equal) with stride-0 broadcast access patterns.
  - PSUM ps1 [128, 512] accumulates a supertile as ONE accumulation group
    (start on first matmul, stop on last; lazy bank zeroing).
  - recip arrives as [1, 5056] bf16, broadcast across partitions via a
    K=1 ones-matmul; h = ps1 * recip is one wide DVE multiply.
  - Stage 2 per supertile: ps2 = W_selfT.T @ featT + W_neighT.T @ h,
    ACT relu+bias, bf16 DMA out; host un-permutes and upcasts.
  - Emission is software-pipelined (sel build g+1 is queued on the
    in-order Vector sequencer BEFORE the h-mult of g) so DVE overlaps PE.
"""

import heapq

import numpy as np
import ml_dtypes

import concourse.bacc as bacc
import concourse.bass as bass
import concourse.mybir as mybir
import concourse.tile as tile
from concourse.bass_utils import run_bass_kernel_spmd

FP8 = ml_dtypes.float8_e4m3fn
BF16 = ml_dtypes.bfloat16
P = 128
W = 32                 # dst-tile width (columns)
NCORES = 8
SUP = 16               # tiles per supertile (512 columns)

f32 = mybir.dt.float32
bf16 = mybir.dt.bfloat16
fp8 = mybir.dt.float8e4

LAST = {}


def _pack_core(deg, TPC):
    """LPT pack nodes (local ids 0..n-1) into TPC bins of W slots, balancing
    degree sums.  Returns perm [TPC*W] of local ids (-1 = empty)."""
    n = len(deg)
    order = np.argsort(-deg, kind="stable")
    heap = [(0, b) for b in range(TPC)]
    heapq.heapify(heap)
    counts = np.zeros(TPC, np.int64)
    perm = np.full((TPC, W), -1, np.int64)
    for i in order:
        d = int(deg[i])
        while True:
            s, b = heapq.heappop(heap)
            if counts[b] < W:
                break
        perm[b, counts[b]] = i
        counts[b] += 1
        if counts[b] < W:
            heapq.heappush(heap, (s + d, b))
    return perm.reshape(-1)


def _make_plan(feat, src, dst):
    N, D = feat.shape
    assert D == P and N % NCORES == 0
    NPC = N // NCORES
    # extra bins beyond ceil(NPC/W) give LPT headroom so per-tile degree
    # sums stay under a multiple of 128 across all cores (template caps)
    TPC = (NPC + W - 1) // W + 3
    TPC += (-TPC) % SUP  # whole supertiles
    NPAD = TPC * W

    deg = np.bincount(dst, minlength=N)
    recip = (1.0 / np.maximum(deg, 1)).astype(np.float32)

    perms = []
    sums = np.zeros((NCORES, TPC), np.int64)
    for m in range(NCORES):
        lperm = _pack_core(deg[m * NPC : (m + 1) * NPC], TPC)
        perm = np.where(lperm >= 0, lperm + m * NPC, -1)
        perms.append(perm)
        d = np.where(perm >= 0, deg[np.clip(perm, 0, N - 1)], 0)
        sums[m] = d.reshape(TPC, W).sum(1)

    caps = np.maximum(1, np.ceil(sums / 128.0).astype(np.int64)).max(0)  # [TPC]
    coff = np.zeros(TPC + 1, np.int64)
    np.cumsum(caps, out=coff[1:])
    C_TOT = int(coff[-1])

    # supertile sizes: full SUP except the tail, which is split finer to
    # shorten the end-of-kernel serial chain
    sizes = []
    rem = TPC
    while rem > 2 * SUP:
        sizes.append(SUP)
        rem -= SUP
    while rem > 0:
        s = min(SUP // 2, rem)
        sizes.append(s)
        rem -= s
    sups = []
    t0 = 0
    for sz in sizes:
        t1 = t0 + sz
        sups.append(
            dict(t0=t0, t1=t1, c0=int(coff[t0]), c1=int(coff[t1]),
                 col0=t0 * W, col1=t1 * W)
        )
        t0 = t1

    feat8 = feat.astype(FP8)
    core_of = dst // NPC
    tab_all, drel_all, featw_all, recip1_all = [], [], [], []
    for m in range(NCORES):
        perm = perms[m]
        colof = np.full(N, -1, np.int64)
        tileof = np.full(N, -1, np.int64)
        valid = perm >= 0
        colof[perm[valid]] = np.arange(NPAD)[valid] % W
        tileof[perm[valid]] = np.arange(NPAD)[valid] // W

        em = core_of == m
        es = src[em]
        ed = dst[em]
        et = tileof[ed]
        ecol = colof[ed]

        order = np.argsort(et, kind="stable")
        es, et, ecol = es[order], et[order], ecol[order]
        tile_cnt = np.bincount(et, minlength=TPC)
        tile_start = np.zeros(TPC + 1, np.int64)
        np.cumsum(tile_cnt, out=tile_start[1:])

        slot_src = np.zeros(C_TOT * P, np.int64)
        slot_col = np.full(C_TOT * P, -1.0, np.float32)
        for t in range(TPC):
            cnt = int(tile_cnt[t])
            assert cnt <= caps[t] * P, (m, t, cnt, caps[t] * P)
            s0 = int(coff[t]) * P
            sl = slice(tile_start[t], tile_start[t + 1])
            slot_src[s0 : s0 + cnt] = es[sl]
            slot_col[s0 : s0 + cnt] = ecol[sl]

        rows = feat8[slot_src]                       # [C_TOT*128, 128]
        rows[slot_col.reshape(-1) < 0] = 0
        tabw = np.ascontiguousarray(
            rows.reshape(C_TOT, P, P).transpose(1, 0, 2).reshape(P, C_TOT * P)
        )
        drelw = np.ascontiguousarray(slot_col.reshape(C_TOT, P).T).astype(BF16)

        fperm = np.zeros((NPAD, P), np.float32)
        fperm[valid] = feat[perm[valid]]
        featw = np.ascontiguousarray(fperm.T).astype(BF16)

        r1 = np.ones(NPAD, np.float32)
        r1[valid] = recip[perm[valid]]
        recip1_all.append(r1.reshape(1, NPAD).astype(BF16))

        tab_all.append(tabw)
        drel_all.append(drelw)
        featw_all.append(featw)

    plan = dict(N=N, NPC=NPC, TPC=TPC, NPAD=NPAD, caps=caps, coff=coff,
                C_TOT=C_TOT, sups=sups, perms=perms)
    return plan, tab_all, drel_all, featw_all, recip1_all


def _build(plan):
    NPAD = plan["NPAD"]
    caps = plan["caps"]
    coff = plan["coff"]
    C_TOT = plan["C_TOT"]
    sups = plan["sups"]
    G = len(sups)

    CG_MAX = max(s["c1"] - s["c0"] for s in sups)

    nc = bacc.Bacc("TRN2", target_bir_lowering=False, debug=False,
                   num_devices=NCORES)

    tab_t = nc.dram_tensor("tabw", [P, C_TOT * P], fp8, kind="ExternalInput")
    drel_t = nc.dram_tensor("drelw", [P, C_TOT], bf16, kind="ExternalInput")
    featw_t = nc.dram_tensor("featw", [P, NPAD], bf16, kind="ExternalInput")
    recip1_t = nc.dram_tensor("recip1", [1, NPAD], bf16, kind="ExternalInput")
    ones_t = nc.dram_tensor("ones1", [1, P], bf16, kind="ExternalInput")
    wsT_t = nc.dram_tensor("wsT", [P, P], bf16, kind="ExternalInput")
    wnT_t = nc.dram_tensor("wnT", [P, P], bf16, kind="ExternalInput")
    bias_t = nc.dram_tensor("bias", [P, 1], f32, kind="ExternalInput")
    iota_t = nc.dram_tensor("iota16", [P, W], bf16, kind="ExternalInput")
    out_t = nc.dram_tensor("out", [P, NPAD], bf16, kind="ExternalOutput")

    with tile.TileContext(nc) as tc:
        with (
            tc.tile_pool(name="const", bufs=1) as cpool,
            tc.tile_pool(name="tab", bufs=4) as tpool,
            tc.tile_pool(name="sel", bufs=16) as spool,
            tc.tile_pool(name="hb", bufs=3) as hpool,
            tc.tile_pool(name="rdg", bufs=6) as rpool,
            tc.tile_pool(name="ps1", bufs=3, space="PSUM") as p1pool,
            tc.tile_pool(name="psr", bufs=3, space="PSUM") as prpool,
            tc.tile_pool(name="ps2", bufs=2, space="PSUM") as p2pool,
        ):
            drel_sb = cpool.tile([P, C_TOT], bf16, tag="drel")
            recip1_sb = cpool.tile([1, NPAD], bf16, tag="recip1")
            ones_sb = cpool.tile([1, P], bf16, tag="ones")
            featw_sb = cpool.tile([P, NPAD], bf16, tag="featw")
            out_sb = cpool.tile([P, NPAD], bf16, tag="out")
            wsT_sb = cpool.tile([P, P], bf16, tag="ws")
            wnT_sb = cpool.tile([P, P], bf16, tag="wn")
            bias_sb = cpool.tile([P, 1], f32, tag="bias")
            iota_sb = cpool.tile([P, W], bf16, tag="iota")

            # drel first on the bulk queue (gates the first sel build);
            # small consts on sync/scalar queues
            nc.gpsimd.dma_start(drel_sb[:], drel_t.ap()[:])
            nc.sync.dma_start(iota_sb[:], iota_t.ap()[:])
            nc.scalar.dma_start(recip1_sb[:], recip1_t.ap()[:])
            nc.scalar.dma_start(ones_sb[:], ones_t.ap()[:])
            nc.scalar.dma_start(wsT_sb[:], wsT_t.ap()[:])
            nc.scalar.dma_start(wnT_sb[:], wnT_t.ap()[:])
            nc.scalar.dma_start(bias_sb[:], bias_t.ap()[:])

            tabs = []
            for g, s in enumerate(sups):
                tg = tpool.tile([P, CG_MAX * P], fp8, tag="tab")
                nc.gpsimd.dma_start(
                    tg[:, : (s["c1"] - s["c0"]) * P],
                    tab_t.ap()[:, s["c0"] * P : s["c1"] * P],
                )
                nc.gpsimd.dma_start(
                    featw_sb[:, s["col0"] : s["col1"]],
                    featw_t.ap()[:, s["col0"] : s["col1"]],
                )
                tabs.append(tg)

            sels = [None] * G
            ps1s = [None] * G
            rdgs = [None] * G

            def emit_rbc(g):
                s = sups[g]
                col0, col1 = s["col0"], s["col1"]
                GW = col1 - col0
                psr = prpool.tile([P, 512], f32, tag="psr")
                nc.tensor.matmul(
                    psr[:, :GW], lhsT=ones_sb[:], rhs=recip1_sb[:, col0:col1],
                    start=True, stop=True,
                )
                rdg = rpool.tile([P, 512], bf16, tag="rdg")
                nc.scalar.activation(
                    rdg[:, :GW], psr[:, :GW],
                    mybir.ActivationFunctionType.Copy,
                )
                rdgs[g] = rdg

            def emit_front(g):
                s = sups[g]
                c0, c1 = s["c0"], s["c1"]
                col0, col1 = s["col0"], s["col1"]
                CG = c1 - c0
                GW = col1 - col0
                # wide sel build
                selg = spool.tile([P, CG_MAX * W], fp8, tag="sel")
                io = iota_sb[:]
                in0 = bass.AP(io.tensor, io.offset,
                              [[io.ap[0][0], P], [0, CG], [1, W]])
                dr = drel_sb[:, c0:c1]
                in1 = bass.AP(dr.tensor, dr.offset,
                              [[dr.ap[0][0], P], [1, CG], [0, W]])
                nc.vector.tensor_tensor(
                    out=selg[:, : CG * W], in0=in0, in1=in1,
                    op=mybir.AluOpType.is_equal,
                )
                sels[g] = selg

            def emit_scatter(g):
                s = sups[g]
                t0, t1 = s["t0"], s["t1"]
                c0 = s["c0"]
                selg = sels[g]
                tg = tabs[g]
                ps1 = p1pool.tile([P, 512], f32, tag="ps1")
                n_mm = int(coff[t1] - coff[t0])
                k = 0
                for t in range(t0, t1):
                    ti = t - t0
                    cg0 = int(coff[t]) - c0
                    out_ap = ps1[:, ti * W : (ti + 1) * W]
                    for c in range(int(caps[t])):
                        cg = cg0 + c
                        nc.tensor.matmul(
                            out_ap,
                            lhsT=tg[:, cg * P : (cg + 1) * P],
                            rhs=selg[:, cg * W : (cg + 1) * W],
                            start=(k == 0), stop=(k == n_mm - 1),
                            skip_group_check=True,
                        )
                        k += 1
                ps1s[g] = ps1

            hbs = [None] * G



            def emit_mult(g):
                s = sups[g]
                col0, col1 = s["col0"], s["col1"]
                GW = col1 - col0
                hb = hpool.tile([P, 512], bf16, tag="hb")
                nc.vector.tensor_tensor(
                    out=hb[:, :GW], in0=ps1s[g][:, :GW],
                    in1=rdgs[g][:, :GW],
                    op=mybir.AluOpType.mult,
                )
                hbs[g] = hb

            def emit_stage2(g):
                s = sups[g]
                col0, col1 = s["col0"], s["col1"]
                GW = col1 - col0
                ps2 = p2pool.tile([P, 512], f32, tag="ps2")
                nc.tensor.matmul(ps2[:, :GW], lhsT=wsT_sb[:],
                                 rhs=featw_sb[:, col0:col1],
                                 start=True, stop=False)
                nc.tensor.matmul(ps2[:, :GW], lhsT=wnT_sb[:],
                                 rhs=hbs[g][:, :GW],
                                 start=False, stop=True)
                nc.scalar.activation(
                    out_sb[:, col0:col1], ps2[:, :GW],
                    mybir.ActivationFunctionType.Relu,
                    bias=bias_sb[:, 0:1],
                )
                nc.sync.dma_start(out_t.ap()[:, col0:col1],
                                  out_sb[:, col0:col1])

            # all sel builds in a prologue (they only need drel/iota and
            # ~1KB/partition each, and always finish ahead of the table
            # stream), so the main loop's Vector stream is just the MULTs:
            #   PE:     scatter0, scatter1, s2_0, scatter2, s2_1, ...
            #   Vector: IS_EQ x G, MULT0, MULT1, ...
            emit_front(0)
            if G > 1:
                emit_front(1)
            if G > 2:
                emit_front(2)
            emit_rbc(0)
            if G > 1:
                emit_rbc(1)
            for g in range(G):
                if g >= 1:
                    emit_mult(g - 1)
                if g + 2 < G:
                    emit_rbc(g + 2)
                emit_scatter(g)
                if g + 3 < G:
                    emit_front(g + 3)
                if g >= 1:
                    emit_stage2(g - 1)
            emit_mult(G - 1)
            emit_stage2(G - 1)

    nc.compile()
    return nc


def kernel(feat, src, dst, W_self, b_self, W_neigh, b_neigh):
    feat = np.asarray(feat, np.float32)
    src = np.asarray(src, np.int64)
    dst = np.asarray(dst, np.int64)
    N, D = feat.shape

    plan, tab_all, drel_all, featw_all, recip1_all = _make_plan(feat, src, dst)

    wsT = np.ascontiguousarray(np.asarray(W_self, np.float32).T).astype(BF16)
    wnT = np.ascontiguousarray(np.asarray(W_neigh, np.float32).T).astype(BF16)
    bias = (
        (np.asarray(b_self, np.float32) + np.asarray(b_neigh, np.float32))
        .astype(np.float32).reshape(P, 1)
    )
    iota16 = np.ascontiguousarray(
        np.broadcast_to(np.arange(W, dtype=np.float32), (P, W))
    ).astype(BF16)
    ones1 = np.ones((1, P), BF16)

    in_maps = []
    for m in range(NCORES):
        in_maps.append(
            dict(tabw=tab_all[m], drelw=drel_all[m], featw=featw_all[m],
                 recip1=recip1_all[m], ones1=ones1, wsT=wsT, wnT=wnT, bias=bias,
                 iota16=iota16)
        )

    key = (N, D, plan["C_TOT"], plan["caps"].tobytes())
    if LAST.get("key") != key:
        nc = _build(plan)
        LAST.update(key=key, nc=nc)
    nc = LAST["nc"]
    LAST["in_maps"] = in_maps

    res = run_bass_kernel_spmd(nc, in_maps, core_ids=list(range(NCORES)))

    out = np.empty((N, P), np.float32)
    for m in range(NCORES):
        o = np.asarray(res.results[m]["out"]).astype(np.float32).T
        perm = plan["perms"][m]
        valid = perm >= 0
        out[perm[valid]] = o[valid]
    return np.ascontiguousarray(out)
